# revision 70
# baseline (speedup 1.0000x reference)
"""Trainium2 Bass kernel for nn_AttentionPermMatrix (Sinkhorn permutation sampling).

Contract: kernel(b_q, b_k, gumbel_u) takes FULL inputs
  b_q, b_k: [64, 128, 64, 64] f32, gumbel_u: [64, 64, 64] f32
and returns the FULL output [64, 64, 64] f32.

Strategy: pure data-parallel over B=64 (8 slices per NeuronCore, 8 cores).
v3 design (vs v2 baseline @82us):
  - v2 was LDWEIGHTS-bandwidth-bound: the 512 mean-pool matmuls loaded the
    q/k data as 128x128 fp8 *stationaries* (~104ns each = 53us of weight
    streaming at ~157B/ns). v3 streams the data as the *moving* operand in
    fp8 DoubleRow mode: per slice-tensor, 4 matmuls contract k-tile pairs
    (chunk c, chunk 4+c) of a [128, 2, 512] view against a tiny [128,2,8]
    fp8 routing stationary (ones at (0,c),(1,4+c)) -> psum [8,512] = the 8
    chunk-sum rows, at partition 0 (BIR requires matmul psum partition 0).
  - psum -> sbuf staging copy on the (otherwise idle) ACT engine (AF.Copy
    shares the ln/exp act table -> no table reloads), then ONE reshape DMA
    [8,512]->[64,64] per tensor gives mean tiles U with partition=e.
  - R = Uq^T Uk via ONE f32 matmul per slice (64-partition contraction)
    into pr [64,2,64]; ln-poly/gumbel/exp/row-normalize prep batches both
    slices of a group per DVE op. Per-slice [64,64] stationaries qm (=Q)
    and pt (=Q^T, via PE transpose) in f16+f32 replace v2's 128x128
    block-diag assembly (8KB f16 LDWs instead of 32KB).
  - Sinkhorn chain: two 2-group chains. Per half-step: 4 matvecs (one per
    slice) into adjacent psum cols + ONE batched [64,4] reciprocal.
    chainX (slices 0-3) overlaps the tail of the input DMA; chainY
    (slices 4-7) is the exposed tail.
  - output: fold u into qm rows (per-partition scale), PE transpose, scale
    by v -> osT stored transposed; host unpermutes (free). The u-phase
    (scales + transposes) is emitted before the chain's final half-step so
    it overlaps; only the v-scale + stores (split across both DMA rings)
    follow the last reciprocal (~2us tail instead of ~6us).
  - static-schedule control: tile_wait_until floors on the means encode
    measured DMA arrival times; the per-engine counting semaphores make
    every op transitively wait on all earlier same-engine ops in static
    order, so scheduler misordering (from its optimistic DMA model)
    otherwise couples the latency-bound chain to the data-gated means.
"""
import math
from contextlib import ExitStack

import numpy as np
import ml_dtypes

import concourse.bass as bass
import concourse.tile as tile
from concourse import bacc, mybir
from concourse.bass_utils import run_bass_kernel_spmd
from concourse.masks import make_identity

F32 = mybir.dt.float32
F16 = mybir.dt.float16
FP8 = mybir.dt.float8e4
AF = mybir.ActivationFunctionType
AX = mybir.AxisListType
OP = mybir.AluOpType
DR = mybir.MatmulPerfMode.DoubleRow

BLOCK, E, BLOCKS = 128, 64, 64
FB = E * BLOCKS              # 4096 flattened (e, i)
TEMP = 0.7
N_ITERS = 8
EPS = 1e-6
# pm holds raw column SUMS; R' = sums_dot * 128^-2 * 128^-0.5
# ln(R') = ln(sqrt(2)) + ln1p(R'/sqrt(2) - 1) via DVE polynomial; constants
# fold into the gumbel bias.
C_TOT = 0.5 * math.log(2.0)
R_SCALE = float(BLOCK) ** -2.5 / math.sqrt(2.0)
N_CORES = 8
NG = 4                       # groups of 2 slices


def emit(tc, q, k, g, out, S):
    nc = tc.nc
    assert S == 8
    with ExitStack() as ctx:
        ctx.enter_context(nc.allow_low_precision(
            reason="fp8 inputs + f16 chain; validated vs 2e-2 gate"))
        consts = ctx.enter_context(tc.tile_pool(name="consts", bufs=1))
        qk = ctx.enter_context(tc.tile_pool(name="qk", bufs=2 * S))
        glob = ctx.enter_context(tc.tile_pool(name="glob", bufs=1))
        stg = ctx.enter_context(tc.tile_pool(name="stg", bufs=4))
        up = ctx.enter_context(tc.tile_pool(name="up", bufs=NG))
        grp = ctx.enter_context(tc.tile_pool(name="grp", bufs=4))
        uvp = ctx.enter_context(tc.tile_pool(name="uvp", bufs=8))
        outp = ctx.enter_context(tc.tile_pool(name="outp", bufs=4))
        ps = ctx.enter_context(tc.tile_pool(name="ps", bufs=1, space="PSUM"))

        ident = consts.tile([128, 128], F32)
        make_identity(nc, ident)
        eps_col = consts.tile([64, 1], F32)
        nc.vector.memset(eps_col, EPS)
        ones16 = consts.tile([64, 1], F16)
        nc.vector.memset(ones16, 1.0)
        # DoubleRow routing stationaries: es[c][k, 0, c] = 1, es[c][k, 1, 4+c] = 1
        # (16 cols per k-tile: DoubleRow LDWEIGHTS rejects 8-wide stationaries;
        # psum rows 8:15 stay zero)
        ess = []
        for c in range(4):
            es = consts.tile([128, 2, 16], FP8, name=f"es{c}")
            nc.vector.memset(es, 0.0)
            nc.vector.memset(es[:, 0, c:c + 1], 1.0)
            nc.vector.memset(es[:, 1, 4 + c:5 + c], 1.0)
            ess.append(es)

        # q/k bulk loads FIRST: slice-PAIR DMAs (1MB each, 8KB contiguous per
        # partition; host pre-interleaves pairs) split over both hwdge rings
        # (sync=q, scalar=k). k is emitted before any ACT work so the scalar
        # queue dispatches all four k DMAs back-to-back.
        # Tile view [128, 2(slice), 2(ktile-pair), 4(c), 512]: dim2 pairs
        # chunk c with chunk 4+c at stride 2048.
        qt2s, kt2s = [], []
        for sp in range(S // 2):
            qt = qk.tile([BLOCK, 2, 2, 4, 512], FP8, tag="qt", bufs=S // 2,
                         name=f"qt{sp}")
            nc.sync.dma_start(out=qt[:], in_=q.ap()[sp])
            qt2s.append(qt)
        for sp in range(S // 2):
            kt = qk.tile([BLOCK, 2, 2, 4, 512], FP8, tag="kt", bufs=S // 2,
                         name=f"kt{sp}")
            nc.scalar.dma_start(out=kt[:], in_=k.ap()[sp])
            kt2s.append(kt)

        def qk_src(s, t):
            t2 = (qt2s if t == 0 else kt2s)[s // 2]
            return t2[:, s % 2, :, :, :]

        # gumbel prologue: hb = C_TOT - ln(-ln(u+eps)+eps); g is [64, S, 64]
        # (loaded on the gpsimd SWDGE ring to keep both hwdge rings free for
        # the bulk q/k input stream)
        gt = glob.tile([64, S, BLOCKS], F32)
        nc.gpsimd.dma_start(out=gt, in_=g.ap())
        ga = glob.tile([64, S, BLOCKS], F32)
        nc.scalar.activation(ga, gt, AF.Ln, bias=eps_col[:], scale=1.0)
        gb = glob.tile([64, S, BLOCKS], F32)
        nc.scalar.activation(gb, ga, AF.Ln, bias=eps_col[:], scale=-1.0)
        hb = glob.tile([64, S, BLOCKS], F32)
        nc.vector.tensor_scalar(out=hb, in0=gb, scalar1=-1.0, scalar2=C_TOT,
                                op0=OP.mult, op1=OP.add)
        osT = glob.tile([64, S, BLOCKS], F32)

        pms = {}

        def means(s, t):
            """4 DoubleRow MMs: slice s tensor t -> psum [8, 512] chunk sums."""
            pm = ps.tile([16, 512], F32, tag="pm", bufs=4, name=f"pm{s}_{t}")
            pms[(s, t)] = pm
            thunks = []
            src = qk_src(s, t)
            for c in range(4):
                def mm(c=c, pm=pm, src=src):
                    nc.tensor.matmul(pm[:], lhsT=ess[c][:],
                                     rhs=src[:, :, c, :],
                                     start=(c == 0), stop=(c == 3),
                                     perf_mode=DR, skip_group_check=True)
                thunks.append(mm)
            return thunks

        up_g = [None] * NG

        def extract(s, t):
            """psum [8,512] -> sbuf f16 (ACT engine) -> reshape DMA -> U."""
            st = stg.tile([8, 512], F16, tag="stg", name=f"stg{s}_{t}")
            nc.scalar.activation(st[:], pms[(s, t)][0:8, :], AF.Copy, scale=1.0)
            gi, h = s // 2, s % 2
            if up_g[gi] is None:
                up_g[gi] = up.tile([64, 4, 64], F16, tag="U", name=f"U{gi}")
            # pair-3 extracts gate chainY: use the sync HWDGE ring (idle once
            # inputs drain, ~1us faster than the gpsimd SWDGE path)
            ring = nc.sync if s >= 6 else nc.gpsimd
            ring.dma_start(out=up_g[gi][:, 2 * h + t, :], in_=st[:])

        prpts = {}
        prs = {}

        def rprep(gi):
            """PE: R_A -> pr[:,0,:], R_B -> pr[:,1,:] (64-contraction, f32)."""
            prpt = ps.tile([64, 4, 64], F32, tag="prpt", bufs=2,
                           name=f"prpt{gi}")
            prpts[gi] = prpt
            U = up_g[gi]
            nc.tensor.matmul(prpt[:, 0, :], lhsT=U[:, 0, :], rhs=U[:, 1, :],
                             start=True, stop=True, skip_group_check=True)
            nc.tensor.matmul(prpt[:, 1, :], lhsT=U[:, 2, :], rhs=U[:, 3, :],
                             start=True, stop=True, skip_group_check=True)
            prs[gi] = prpt[:, 0:2, :]

        qm_g, qmh_g, ptf_g, pth_g = [None] * NG, [None] * NG, [None] * NG, [None] * NG
        u1_g = [None] * NG

        def prep_a(gi, eng):
            """ln-poly + gumbel bias + exp on [64, 2, 64] tiles."""
            pr = prs[gi]
            xg = grp.tile([64, 2, 64], F32, tag="xg", name=f"xg{gi}")
            eng.tensor_scalar(out=xg[:], in0=pr, scalar1=R_SCALE,
                              scalar2=-1.0, op0=OP.mult, op1=OP.add)
            pl = grp.tile([64, 2, 64], F32, tag="pl", name=f"pl{gi}")
            eng.tensor_scalar(out=pl[:], in0=xg[:], scalar1=-0.25,
                              scalar2=1.0 / 3.0, op0=OP.mult, op1=OP.add)
            eng.tensor_tensor(out=pl[:], in0=pl[:], in1=xg[:], op=OP.mult)
            eng.tensor_scalar(out=pl[:], in0=pl[:], scalar1=-0.5,
                              scalar2=None, op0=OP.add)
            eng.tensor_tensor(out=pl[:], in0=pl[:], in1=xg[:], op=OP.mult)
            eng.tensor_scalar(out=pl[:], in0=pl[:], scalar1=1.0,
                              scalar2=None, op0=OP.add)
            eng.tensor_tensor(out=pl[:], in0=pl[:], in1=xg[:], op=OP.mult)
            ts = grp.tile([64, 2, 64], F32, tag="ts", name=f"ts{gi}")
            eng.tensor_add(ts[:], pl[:], hb[:, 2 * gi:2 * gi + 2, :])
            p0 = grp.tile([64, 2, 64], F32, tag="p0", name=f"p0{gi}")
            nc.scalar.activation(p0[:], ts[:], AF.Exp, scale=1.0 / TEMP)
            return p0

        def prep_recip(gi, p0):
            """rowsums + reciprocal, always on DVE."""
            rs = uvp.tile([64, 2], F32, tag="rs", bufs=4, name=f"rs{gi}")
            nc.vector.reduce_sum(rs[:], p0[:], axis=AX.X)
            u1 = uvp.tile([64, 2], F32, tag="u1", bufs=4, name=f"u1{gi}")
            nc.vector.reciprocal(u1[:], rs[:])
            u1_g[gi] = u1

        def prep_b(gi, p0, eng):
            """fold iteration 1's row-normalize: qm = diag(1/rowsum) P0."""
            qm = grp.tile([64, 2, 64], F32, tag="qm", name=f"qm{gi}")
            u1 = u1_g[gi]
            for h in range(2):
                eng.tensor_scalar(out=qm[:, h, :], in0=p0[:, h, :],
                                  scalar1=u1[:, h:h + 1], scalar2=None,
                                  op0=OP.mult)
            qmh = grp.tile([64, 2, 64], F16, tag="qmh", name=f"qmh{gi}")
            eng.tensor_copy(qmh[:], qm[:])
            qm_g[gi], qmh_g[gi] = qm, qmh

        def qtrans(gi):
            """PE transposes qm slices -> pt psum region prpt[:, 2:4, :]."""
            qm = qm_g[gi]
            pt = prpts[gi][:, 2:4, :]
            nc.tensor.transpose(pt[:, 0, :], qm[:, 0, :], ident[0:64, 0:64])
            nc.tensor.transpose(pt[:, 1, :], qm[:, 1, :], ident[0:64, 0:64])
            return pt

        def qtrans_copy(gi, pt, eng):
            pth = grp.tile([64, 2, 64], F16, tag="pth", name=f"pth{gi}")
            eng.tensor_copy(pth[:], pt)
            pth_g[gi] = pth

        pmv2 = ps.tile([64, 2, 64], F32, tag="pmv", bufs=1, name="pmv")

        def chain(groups, tagc):
            """15 half-steps for 2 groups (4 slices); per half-step 4 matvecs
            into adjacent psum cols + ONE [64,4] reciprocal. All f16
            (stationaries AND states; validated 9.4e-3 vs the 2e-2 gate)."""
            pmv = pmv2[:, 0 if tagc == "X" else 1, :]
            state = {"u": None, "v": None}
            thunks = []
            for it in range(N_ITERS):
                def vstep(it=it):
                    col = 8 * it
                    for j, gi in enumerate(groups):
                        lhs = qmh_g[gi]
                        for h in range(2):
                            rhs = (ones16[:] if it == 0
                                   else state["u"][:, 2 * j + h:2 * j + h + 1])
                            nc.tensor.matmul(
                                pmv[:, col + 2 * j + h:col + 2 * j + h + 1],
                                lhsT=lhs[:, h, :], rhs=rhs,
                                start=True, stop=True, skip_group_check=True)
                    v = uvp.tile([64, 4], F16, tag=f"uv{tagc}",
                                 name=f"v{tagc}_{it}")
                    nc.vector.reciprocal(v[:], pmv[:, col:col + 4])
                    state["v"] = v
                thunks.append(vstep)
                if it < N_ITERS - 1:
                    def ustep(it=it):
                        col = 8 * it + 4
                        for j, gi in enumerate(groups):
                            lhs = pth_g[gi]
                            for h in range(2):
                                nc.tensor.matmul(
                                    pmv[:, col + 2 * j + h:col + 2 * j + h + 1],
                                    lhsT=lhs[:, h, :],
                                    rhs=state["v"][:, 2 * j + h:2 * j + h + 1],
                                    start=True, stop=True,
                                    skip_group_check=True)
                        u = uvp.tile([64, 4], F16, tag=f"uv{tagc}",
                                     name=f"u{tagc}_{it}")
                        nc.vector.reciprocal(u[:], pmv[:, col:col + 4])
                        state["u"] = u
                    thunks.append(ustep)
            return thunks, state

        def out_phase_u(st, groups, tagc):
            """After the final u-step: cast u, scale qm rows, PE transposes.
            Emitted before the chain's final half-step so the transposes
            overlap it; only the v-scale + store remain after the last
            reciprocal (cuts the output tail from ~6us to ~2us)."""
            uf = uvp.tile([64, 4], F32, tag=f"uvf{tagc}", bufs=2,
                          name=f"uf{tagc}")
            nc.vector.tensor_copy(uf[:], st["u"][:])
            pos = {}
            for j, gi in enumerate(groups):
                qmu = outp.tile([64, 2, 64], F32, tag="qmu", name=f"qmu{gi}")
                for h in range(2):
                    nc.vector.tensor_scalar(
                        out=qmu[:, h, :], in0=qm_g[gi][:, h, :],
                        scalar1=uf[:, 2 * j + h:2 * j + h + 1],
                        scalar2=None, op0=OP.mult)
                po = ps.tile([64, 4, 64], F32, tag="prpt", bufs=2,
                             name=f"po{gi}")
                nc.tensor.transpose(po[:, 0, :], qmu[:, 0, :],
                                    ident[0:64, 0:64])
                nc.tensor.transpose(po[:, 1, :], qmu[:, 1, :],
                                    ident[0:64, 0:64])
                pos[gi] = po
            return pos

        def out_phase_v(st, groups, tagc, pos):
            """After the final v-step: cast v, scale into osT."""
            vf = uvp.tile([64, 4], F32, tag=f"uvf{tagc}", bufs=2,
                          name=f"vf{tagc}")
            nc.vector.tensor_copy(vf[:], st["v"][:])
            for j, gi in enumerate(groups):
                for h in range(2):
                    nc.vector.tensor_scalar(
                        out=osT[:, 2 * gi + h, :], in0=pos[gi][:, h, :],
                        scalar1=vf[:, 2 * j + h:2 * j + h + 1],
                        scalar2=None, op0=OP.mult)

        def arrival_ms(s, t):
            """max(measured ring arrival, contiguous PE-burst pacing).
            Model-time floor only (bass_wait_until_ts) - replaces the static
            scheduler's optimistic DMA estimates with measured truth so the
            PE stream order minimizes coupling stalls (per-engine counting
            semaphores make every PE op transitively wait on all earlier PE
            ops in static order). Contiguous pacing keeps PE bursts back to
            back so the p-state ramps to full clock."""
            return (13.5 + 2.5 * t + 5.0 * (s // 2)) * 1e-3

        def means_extract(s):
            for t in (0, 1):
                with tc.tile_wait_until(arrival_ms(s, t)):
                    for th in means(s, t):
                        th()
                    extract(s, t)

        def group_prep(gi, floor=None):
            """floor (ms) keeps consecutive groups' DVE prep sequences
            disjoint in the static order: without it the scheduler's
            optimistic extract-latency model interleaves them, making one
            group's exp transitively wait the next group's R via the DVE
            counting semaphore."""
            with tc.tile_wait_until(floor * 1e-3 if floor else 0,
                                    enable=floor is not None):
                rprep(gi)
                p0 = prep_a(gi, nc.vector)
                prep_recip(gi, p0)
                prep_b(gi, p0, nc.vector)
                pt = qtrans(gi)
                qtrans_copy(gi, pt, nc.vector)

        # ---- emission: every queue floor-paced to measured runtime so the
        # static order has no cross-group sem-coupling inversions; pair-3
        # means and X-outputs absorb into the chains' recip gaps ----
        for gi in (0, 1):
            means_extract(2 * gi)
            means_extract(2 * gi + 1)
            group_prep(gi)
        for s in (4, 5):
            means_extract(s)
        group_prep(2)
        chX, stX = chain((0, 1), "X")
        for i, th in enumerate(chX):
            with tc.tile_wait_until((25.5 + 0.65 * i) * 1e-3):
                th()
        for s in (6, 7):
            means_extract(s)
        group_prep(3)
        posX = out_phase_u(stX, (0, 1), "X")
        out_phase_v(stX, (0, 1), "X", posX)
        nc.sync.dma_start(out=out.ap()[:, 0:4, :], in_=osT[:, 0:4, :])

        chY, stY = chain((2, 3), "Y")
        for i, th in enumerate(chY[:-1]):
            with tc.tile_wait_until((37.0 + 0.62 * i) * 1e-3):
                th()
        # overlap the output u-phase with the chain's final half-step
        posY = out_phase_u(stY, (2, 3), "Y")
        chY[-1]()
        out_phase_v(stY, (2, 3), "Y", posY)
        nc.sync.dma_start(out=out.ap()[:, 4:6, :], in_=osT[:, 4:6, :])
        nc.scalar.dma_start(out=out.ap()[:, 6:8, :], in_=osT[:, 6:8, :])


def build_nc(S=8):
    nc = bacc.Bacc("TRN2", target_bir_lowering=False, debug=False)
    q = nc.dram_tensor("q", [S // 2, BLOCK, 2 * FB], FP8, kind="ExternalInput")
    k = nc.dram_tensor("k", [S // 2, BLOCK, 2 * FB], FP8, kind="ExternalInput")
    g = nc.dram_tensor("g", [64, S, BLOCKS], F32, kind="ExternalInput")
    out = nc.dram_tensor("out", [64, S, BLOCKS], F32, kind="ExternalOutput")
    with tile.TileContext(nc) as tc:
        emit(tc, q, k, g, out, S)
    nc.compile()
    return nc


_NC_CACHE = {}
LAST_RESULTS = None


def kernel(b_q, b_k, gumbel_u, _trace=False):
    global LAST_RESULTS
    b_q = np.asarray(b_q).astype(ml_dtypes.float8_e4m3fn)
    b_k = np.asarray(b_k).astype(ml_dtypes.float8_e4m3fn)
    gumbel_u = np.asarray(gumbel_u).astype(np.float32)
    B = b_q.shape[0]
    S = B // N_CORES
    if S not in _NC_CACHE:
        _NC_CACHE[S] = build_nc(S)
    nc = _NC_CACHE[S]
    in_maps = []
    for c in range(N_CORES):
        sl = slice(c * S, (c + 1) * S)
        # interleave slice pairs: [S/2, 128, 2*FB] with per-partition rows
        # holding both slices of the pair back-to-back
        qp = (b_q[sl].reshape(S // 2, 2, BLOCK, FB)
              .transpose(0, 2, 1, 3).reshape(S // 2, BLOCK, 2 * FB))
        kp = (b_k[sl].reshape(S // 2, 2, BLOCK, FB)
              .transpose(0, 2, 1, 3).reshape(S // 2, BLOCK, 2 * FB))
        in_maps.append({
            "q": np.ascontiguousarray(qp),
            "k": np.ascontiguousarray(kp),
            "g": np.ascontiguousarray(gumbel_u[sl].transpose(1, 0, 2)),
        })
    res = run_bass_kernel_spmd(nc, in_maps, core_ids=list(range(N_CORES)),
                               trace=_trace)
    LAST_RESULTS = res
    out = np.empty((B, BLOCKS, BLOCKS), dtype=np.float32)
    for c in range(N_CORES):
        oc = res.results[c]["out"]  # [64, S, 64] = (j, s, i)
        for s in range(S):
            out[c * S + s] = oc[:, s, :].T
    return out
